# revision 8
# baseline (speedup 1.0000x reference)
"""Trainium2 Bass kernel for batched per-item GRU cell.

Problem: nn_GRU_Cell — B=16, N=207 independent items, each with its own
C=64 -> 3H=192 weight matrices (Wx, Wh).  All ops are per-(b,n):

    xW          = x @ Wx                      [1, 192]
    r           = sigmoid(xW_r + h @ Wh_r + b_r)
    z           = sigmoid(xW_z + h @ Wh_z + b_z)
    hc          = tanh  (xW_c + (r*h) @ Wh_c + b_c)
    h_new       = (1 - z) * h + z * hc

Strategy (per core, items sharded 3312 -> 8 x 414):
  * Weights stream once from HBM in bf16 (20.3MB/core, measured
    ~395GB/s sustained = ~53us — the roofline).  Per item they are the
    PE *stationary* operand, K-stacked:
      S_rz = [Wx[:, 0:128] ; Wh[:, 0:128]]  (K=128, M=128)
      S_c  = [Wx[:,128:192]; Wh[:,128:192]] (K=128, M=64)
    moving operand is a single bf16 column ([x;h], then [x;r*h]).
    Outputs land as dense PSUM columns [j, item] -> cheap eviction.
  * ALL activations/biases arrive HOST-TRANSPOSED ([c|j, items] blocks)
    in one leading DMA, so the device does zero layout work: the PE
    runs nothing but LDWEIGHTS+MATMUL pairs.
  * DMA orchestration: everything inbound on the SP/sync HWDGE queue
    (aux first, then weight sub-DMAs of <=48 items, one continuous
    burst); h_new stores go out on the scalar HWDGE queue where they
    tuck behind the gate activations.
  * The compute pipeline is sub-granular (48 items; 10 on the tail
    chunk): rz matmuls for a sub, its bias/sigmoid/r*h chain, then its
    c matmuls — the PE trails the DMA stream by ~one sub and almost
    nothing dangles after the last DMA byte.
  * h_new is produced as [h, items] on partitions 64:128 and stored
    as-is; the host does the final [64, G] -> [G, 64] transpose during
    unsharding (layout-only).
"""

import numpy as np

import concourse.bass as bass
import concourse.mybir as mybir
import concourse.tile as tile
from concourse import bacc
from concourse.bass_utils import run_bass_kernel_spmd

F32 = mybir.dt.float32
BF16 = mybir.dt.bfloat16

B, N, C, H = 16, 207, 64, 64
J = 3 * H                  # 192
ITEMS = B * N              # 3312
NCORES = 8
PER = ITEMS // NCORES      # 414
CHUNKS = [96, 96, 96, 96, 30]   # sum = 414
NCHUNK = len(CHUNKS)
GMAX = max(CHUNKS)
SUB = 48                   # sub-granule (weight DMA + compute pipeline)
SUB_LAST = 10              # finer pacing on the tail chunk
AUXB = 3                   # aux blocks per chunk: xh_T | b_rz_T | b_c_T

AF = mybir.ActivationFunctionType


def _subs(k):
    G = CHUNKS[k]
    step = SUB_LAST if k == NCHUNK - 1 else SUB
    return [(a, min(a + step, G)) for a in range(0, G, step)]


def build_nc(wdt=BF16, mdt=BF16):
    """Build the per-core Bass program.

    wdt: dtype of the streamed weights (DMA volume / LDW speed).
    mdt: dtype of the moving operand columns (must pair with wdt for PE).
    """
    # Bacc (not raw Bass): its compile() runs move_matmul_waits_to_ldweights
    # + generate_event_semaphores, which split multi-waits down to the 1-wait
    # ISA limit of PE instructions.
    nc = bacc.Bacc(None)
    # aux, host-transposed and chunk-packed: for chunk k (offset q=3*s_k):
    #   cols [q      , q+G  ): xh_T  [128, G]  (x.T rows 0:64, h.T 64:128)
    #   cols [q+G    , q+2G ): brz_T [128, G]  (br.T rows 0:64, bz.T 64:128)
    #   cols [q+2G   , q+3G ): bc_T  [64, G]   (rows 0:64)
    aux_d = nc.declare_dram_parameter("aux", [128, AUXB * PER], F32,
                                      isOutput=False)
    # weights arrive host-pre-transposed to per-chunk [c, item, j] blocks
    # (flattened): each sub-DMA reads one contiguous run per partition
    w_d = nc.declare_dram_parameter("wxh", [PER * 2 * C * J], wdt,
                                    isOutput=False)
    # h_new, chunk-packed as [chunk, h, item-in-chunk] (host transposes)
    out_d = nc.declare_dram_parameter("out", [NCHUNK, H, GMAX], F32,
                                      isOutput=True)

    cast_rhs = mdt != F32

    with tile.TileContext(nc) as tc:
        with (
            tc.tile_pool(name="const", bufs=1) as cpool,
            tc.tile_pool(name="w", bufs=3) as wpool,
            tc.tile_pool(name="act", bufs=2) as apool,
            tc.tile_pool(name="prz", bufs=2, space="PSUM") as prz_pool,
            tc.tile_pool(name="pc", bufs=2, space="PSUM") as pc_pool,
        ):
            # ---- one leading DMA with every chunk's x/h/b ----------------
            aux_all = cpool.tile([128, AUXB * PER], F32)
            nc.sync.dma_start(out=aux_all[:], in_=aux_d[:, :])

            s = 0
            woff = 0
            for k in range(NCHUNK):
                G = CHUNKS[k]
                q = AUXB * s

                # ---- this chunk's weights (sync queue, sub-DMAs) ---------
                # w[c(0:64) | c(64:128), item, j] = [Wx ; Wh]
                w = wpool.tile([128, GMAX, J], wdt, tag="w")
                wsrc = w_d[woff:woff + 128 * G * J].rearrange(
                    "(c g j) -> c g j", c=128, g=G)
                for a, bb in _subs(k):
                    nc.sync.dma_start(
                        out=w[:, a:bb, :], in_=wsrc[:, a:bb, :],
                    )

                xh = aux_all[:, q:q + G]
                b_rz = aux_all[:, q + G:q + 2 * G]
                b_c = aux_all[0:64, q + 2 * G:q + 3 * G]
                if cast_rhs:
                    xh_m = apool.tile([128, G], mdt, tag="xh_m")
                    nc.vector.tensor_copy(xh_m[:], xh[:])
                else:
                    xh_m = xh
                # c-pass moving columns: x half never changes, fill it now
                # (off the rz->sigmoid->r*h critical chain)
                rhs2 = apool.tile([128, G], mdt, tag="rhs2")
                nc.vector.tensor_copy(rhs2[0:64, :], xh_m[0:64, :])

                psum_rz = prz_pool.tile([128, G], F32, tag="rz")
                psum_c = pc_pool.tile([128, G], F32, tag="c")
                t_rz = apool.tile([128, G], F32, tag="t_rz")
                rs = apool.tile([128, G], F32, tag="rs")
                zs = apool.tile([128, G], F32, tag="zs")
                t_c = apool.tile([128, G], F32, tag="t_c")
                hc = apool.tile([128, G], F32, tag="hc")
                e = apool.tile([128, G], F32, tag="e")
                f = apool.tile([128, G], F32, tag="f")
                hn = apool.tile([128, G], F32, tag="hn")

                # ---- sub-granular pipeline: rz -> gates -> c -------------
                for a, bb in _subs(k):
                    for g in range(a, bb):
                        nc.tensor.matmul(
                            psum_rz[:, g:g + 1],
                            w[:, g, 0:128],
                            xh_m[:, g:g + 1],
                            start=True, stop=True,
                        )
                    nc.vector.tensor_add(
                        t_rz[:, a:bb], psum_rz[:, a:bb], b_rz[:, a:bb])
                    # r evicted to rows 64:128 so r*h aligns with h there
                    nc.scalar.activation(
                        rs[64:128, a:bb], t_rz[0:64, a:bb], AF.Sigmoid)
                    nc.scalar.activation(
                        zs[64:128, a:bb], t_rz[64:128, a:bb], AF.Sigmoid)
                    nc.vector.tensor_mul(
                        rhs2[64:128, a:bb], rs[64:128, a:bb], xh[64:128, a:bb])
                    for g in range(a, bb):
                        nc.tensor.matmul(
                            psum_c[0:64, g:g + 1],
                            w[:, g, 128:192],
                            rhs2[:, g:g + 1],
                            start=True, stop=True,
                        )
                    # ---- epilogue: hc, h_new = h + z*(hc - h) ------------
                    nc.vector.tensor_add(
                        t_c[0:64, a:bb], psum_c[0:64, a:bb], b_c[:, a:bb])
                    nc.scalar.activation(
                        hc[64:128, a:bb], t_c[0:64, a:bb], AF.Tanh)
                    nc.vector.tensor_sub(
                        e[64:128, a:bb], hc[64:128, a:bb], xh[64:128, a:bb])
                    nc.vector.tensor_mul(
                        f[64:128, a:bb], zs[64:128, a:bb], e[64:128, a:bb])
                    nc.vector.tensor_add(
                        hn[64:128, a:bb], xh[64:128, a:bb], f[64:128, a:bb])

                # ---- store h_new as [h, items]; host transposes ----------
                # scalar HWDGE: tucks behind the gate ACTs, cheap fixed cost
                nc.scalar.dma_start(out=out_d[k, :, 0:G], in_=hn[64:128, 0:G])

                s += G
                woff += 128 * G * J

    nc.compile()
    return nc


_CACHE = {}


def _get_nc(wdt, mdt):
    key = (wdt, mdt)
    if key not in _CACHE:
        _CACHE[key] = build_nc(wdt, mdt)
    return _CACHE[key]


def _shards(x, state, Wx, Wh, b, wdt_np):
    x2 = np.asarray(x, np.float32).reshape(ITEMS, C)
    h2 = np.asarray(state, np.float32).reshape(ITEMS, H)
    b2 = np.asarray(b, np.float32).reshape(ITEMS, J)
    aux2 = np.ascontiguousarray(np.concatenate([x2, h2, b2], axis=1))
    wx2 = np.asarray(Wx).reshape(ITEMS, C, J)
    wh2 = np.asarray(Wh).reshape(ITEMS, H, J)
    w2 = np.concatenate([wx2, wh2], axis=1).astype(wdt_np)
    w2 = w2.reshape(NCORES, PER, 2 * C, J)
    aux3 = aux2.reshape(NCORES, PER, 2 * C + J)
    maps = []
    for i in range(NCORES):
        # aux host-transposed, chunk-packed: xh_T | brz_T | bc_T per chunk
        auxp = np.zeros((128, AUXB * PER), np.float32)
        s = 0
        for k, G in enumerate(CHUNKS):
            q = AUXB * s
            blockt = aux3[i, s:s + G].T          # [320, G]
            auxp[:, q:q + G] = blockt[0:128]          # x | h
            auxp[:, q + G:q + 2 * G] = blockt[128:256]    # br | bz
            auxp[0:64, q + 2 * G:q + 3 * G] = blockt[256:320]  # bc
            s += G
        # per chunk: [items, c, j] -> [c, item-in-chunk, j], flattened
        blocks = []
        s = 0
        for G in CHUNKS:
            blocks.append(w2[i, s:s + G].transpose(1, 0, 2).ravel())
            s += G
        maps.append({"aux": auxp, "wxh": np.concatenate(blocks)})
    return maps


def kernel(x, state, Wx, Wh, b, _trace=False, _wdt=BF16, _mdt=BF16):
    import ml_dtypes
    wdt_np = np.float32 if _wdt == F32 else ml_dtypes.bfloat16
    nc = _get_nc(_wdt, _mdt)
    in_maps = _shards(x, state, Wx, Wh, b, wdt_np)
    res = run_bass_kernel_spmd(nc, in_maps, list(range(NCORES)), trace=_trace)
    # out: [NCHUNK, H, GMAX] per core, chunk-packed -> [ITEMS, H]
    out = np.empty((ITEMS, H), np.float32)
    for i in range(NCORES):
        o = res.results[i]["out"]
        s = 0
        for k, G in enumerate(CHUNKS):
            out[i * PER + s:i * PER + s + G] = o[k, :, 0:G].T
            s += G
    ret = out.reshape(B, N, 1, H)
    if _trace:
        return ret, res
    return ret
